# revision 1
# baseline (speedup 1.0000x reference)
"""Self-attention (8 heads, d=64, B=2, N=4096, D=512) on 8 TRN2 NeuronCores.

Sharding: batch*heads across cores — core c handles batch b=c//4, heads
(2*(c%4), 2*(c%4)+1). Projection weights are sliced per-core on the host;
x is pre-transposed on the host so the device needs no transposes at all.

Device dataflow (per core, fully transposed "scoresT" formulation):
  qT2/kT2 [hd=128, n]  = W.T-chunks @ xT-chunks          (PE, f32r)
  v2      [n, hd+ones] natural                            (PE, bf16 store)
  per head h, per q-chunk qq (1024 wide):
    for kc in 32:  scT psum[128k,1024q] = kh.T @ qh       (PE)
                   attnT = exp(scT*SCALE)  -> bf16 SBUF   (ACT, scale fused)
                   av[65,1024] += v2'[kc].T @ attnT       (PE, accumulate)
    row 64 of av = softmax denominator (ones column of v2')
    outT[h] = av[:64] * (1/denom)                         (DVE + DMA bcast)
  partial[n,512] = sum_h outT[h].T @ woT[h]               (PE)
Host: out[b] = sum of its 4 cores' partials + bo.
"""
import numpy as np
import ml_dtypes
from contextlib import ExitStack

import concourse.bass as bass
from concourse import bacc
import concourse.mybir as mybir
import concourse.tile as tile
from concourse.bass_utils import run_bass_kernel_spmd

B, N, D = 2, 4096, 512
HEADS, DH = 8, 64
SCALE = DH ** -0.5

F32 = mybir.dt.float32
F32R = mybir.dt.bfloat16  # matmul operand dtype (bf16: 1cyc/row, standard path)
BF16 = mybir.dt.bfloat16

QQ_W = 1024          # q-chunk width in the attention loop
N_QQ = N // QQ_W     # 4
N_KC = N // 128      # 32 key chunks
DCH = D // 128       # 4 contraction chunks for projections


def build_bass():
    nc = bacc.Bacc(None, target_bir_lowering=False)

    xT = nc.dram_tensor("xT", [D, N], F32R, kind="ExternalInput")
    wqT = nc.dram_tensor("wqT", [D, 128], F32R, kind="ExternalInput")
    wkT = nc.dram_tensor("wkT", [D, 128], F32R, kind="ExternalInput")
    wvT = nc.dram_tensor("wvT", [D, 128], F32R, kind="ExternalInput")
    woT = nc.dram_tensor("woT", [2, 64, D], F32R, kind="ExternalInput")
    out = nc.dram_tensor("out", [N, D], F32, kind="ExternalOutput")
    recip_dram = nc.dram_tensor("recip_scratch", [N_QQ, 2, QQ_W], F32)

    with tile.TileContext(nc) as tc, ExitStack() as ctx:
        const = ctx.enter_context(tc.tile_pool(name="const", bufs=1))

        # ---- load inputs ----
        xT_sb = const.tile([128, DCH, N], F32R)            # xT[(c p), n] -> [p, c, n]
        nc.sync.dma_start(out=xT_sb, in_=xT.rearrange("(c p) n -> p c n", p=128))
        wq_sb = const.tile([128, DCH, 128], F32R)
        nc.sync.dma_start(out=wq_sb, in_=wqT.rearrange("(c p) m -> p c m", p=128))
        wk_sb = const.tile([128, DCH, 128], F32R)
        nc.sync.dma_start(out=wk_sb, in_=wkT.rearrange("(c p) m -> p c m", p=128))
        wv_sb = const.tile([128, DCH, 128], F32R)
        nc.sync.dma_start(out=wv_sb, in_=wvT.rearrange("(c p) m -> p c m", p=128))
        wo_sb = const.tile([64, 2, D], F32R)
        nc.sync.dma_start(out=wo_sb, in_=woT.rearrange("h d n -> d h n"))

        qT2 = const.tile([128, N], F32R)                   # [2-head d, n]
        kT2 = const.tile([128, N], F32R)
        v2 = const.tile([128, N_KC, 130], BF16)            # [k-part, kc, (v_h0|1|v_h1|1)]
        outT = const.tile([64, 2, N], F32R)                # normalized per-head av

        # ---- projections ----
        with tc.tile_pool(name="proj_psum", bufs=3, space="PSUM") as proj_psum:
            for nt in range(N // 512):
                pq = proj_psum.tile([128, 512], F32, tag="pj")
                for c in range(DCH):
                    nc.tensor.matmul(pq, wq_sb[:, c, :], xT_sb[:, c, bass.ts(nt, 512)],
                                     start=(c == 0), stop=(c == DCH - 1))
                nc.vector.tensor_copy(qT2[:, bass.ts(nt, 512)], pq)
            for nt in range(N // 512):
                pk = proj_psum.tile([128, 512], F32, tag="pj")
                for c in range(DCH):
                    nc.tensor.matmul(pk, wk_sb[:, c, :], xT_sb[:, c, bass.ts(nt, 512)],
                                     start=(c == 0), stop=(c == DCH - 1))
                nc.vector.tensor_copy(kT2[:, bass.ts(nt, 512)], pk)
            # v natural: out[n-tile, hd] = xT-chunk.T @ wv-chunk
            for kc in range(N_KC):
                pv = proj_psum.tile([128, 128], F32, tag="pv")
                for c in range(DCH):
                    nc.tensor.matmul(pv, xT_sb[:, c, bass.ts(kc, 128)], wv_sb[:, c, :],
                                     start=(c == 0), stop=(c == DCH - 1))
                # interleave the two heads' 64-col halves into v2 (cols 0-63, 65-128)
                nc.vector.tensor_copy(v2[:, kc, 0:64], pv[:, 0:64])
                nc.vector.tensor_copy(v2[:, kc, 65:129], pv[:, 64:128])
        # ones columns for the softmax-denominator trick
        nc.vector.memset(v2[:, :, 64], 1.0)
        nc.vector.memset(v2[:, :, 129], 1.0)

        # ---- attention ----
        with (
            tc.tile_pool(name="sc_psum", bufs=2, space="PSUM") as sc_psum,
            tc.tile_pool(name="av_psum", bufs=2, space="PSUM") as av_psum,
            tc.tile_pool(name="attn_sb", bufs=4) as attn_sb,
            tc.tile_pool(name="norm_sb", bufs=2) as norm_sb,
        ):
            for qq in range(N_QQ):
                avs = []
                for h in range(2):
                    av = av_psum.tile([65, QQ_W], F32, tag="av", name=f"av_{qq}_{h}")
                    avs.append(av)
                for kc in range(N_KC):
                    for h in range(2):
                        sc = sc_psum.tile([128, QQ_W], F32, tag="sc", name=f"sc_{qq}_{kc}_{h}")
                        for s in range(QQ_W // 512):
                            nc.tensor.matmul(
                                sc[:, bass.ts(s, 512)],
                                kT2[h * 64:(h + 1) * 64, bass.ts(kc, 128)],
                                qT2[h * 64:(h + 1) * 64, qq * QQ_W + s * 512:qq * QQ_W + (s + 1) * 512],
                                start=True, stop=True)
                        at = attn_sb.tile([128, QQ_W], BF16, tag="at", name=f"at_{qq}_{kc}_{h}")
                        nc.scalar.activation(at, sc, mybir.ActivationFunctionType.Exp,
                                             scale=float(SCALE))
                        for s in range(QQ_W // 512):
                            nc.tensor.matmul(
                                avs[h][:, bass.ts(s, 512)],
                                v2[:, kc, h * 65:(h + 1) * 65],
                                at[:, bass.ts(s, 512)],
                                start=(kc == 0), stop=(kc == N_KC - 1))
                # normalize: outT[h][:, qq] = av[:64] * 1/av[64]
                for h in range(2):
                    av = avs[h]
                    rc = norm_sb.tile([128, QQ_W], F32, tag="rc", name=f"rc_{qq}_{h}")
                    nc.vector.reciprocal(rc[64:65, :], av[64:65, :])
                    bc = norm_sb.tile([64, QQ_W], F32, tag="bc", name=f"bc_{qq}_{h}")
                    nc.sync.dma_start(out=recip_dram[qq:qq + 1, h, :], in_=rc[64:65, :])
                    src = recip_dram[qq, h, :]
                    bcast = bass.AP(tensor=src.tensor, offset=src.offset,
                                    ap=[[0, 64]] + src.ap)
                    nc.sync.dma_start(out=bc, in_=bcast)
                    nc.vector.tensor_mul(outT[:, h, qq * QQ_W:(qq + 1) * QQ_W], av[0:64, :], bc)

        # ---- output projection ----
        with (
            tc.tile_pool(name="op_psum", bufs=3, space="PSUM") as op_psum,
            tc.tile_pool(name="op_sb", bufs=3) as op_sb,
        ):
            for nt in range(N // 128):
                po = op_psum.tile([128, D], F32, tag="po")
                nc.tensor.matmul(po, outT[:, 0, bass.ts(nt, 128)], wo_sb[:, 0, :],
                                 start=True, stop=False)
                nc.tensor.matmul(po, outT[:, 1, bass.ts(nt, 128)], wo_sb[:, 1, :],
                                 start=False, stop=True)
                ob = op_sb.tile([128, D], F32, tag="ob")
                nc.vector.tensor_copy(ob, po)
                nc.sync.dma_start(out=out[bass.ts(nt, 128), :], in_=ob)

    nc.compile()
    return nc


_NC_CACHE = None


def build_in_maps(x, Wq, Wk, Wv, Wo):
    bf = ml_dtypes.bfloat16
    x = np.asarray(x, np.float32)
    Wq, Wk, Wv, Wo = (np.asarray(a, np.float32) for a in (Wq, Wk, Wv, Wo))
    in_maps = []
    for c in range(8):
        b = c // 4
        h0 = 2 * (c % 4)
        xT = np.ascontiguousarray(x[b].T.astype(bf))
        wqT = np.ascontiguousarray(Wq[h0 * 64:(h0 + 2) * 64].T.astype(bf))
        wkT = np.ascontiguousarray(Wk[h0 * 64:(h0 + 2) * 64].T.astype(bf))
        wvT = np.ascontiguousarray(Wv[h0 * 64:(h0 + 2) * 64].T.astype(bf))
        woT = np.stack([np.ascontiguousarray(Wo[:, (h0 + h) * 64:(h0 + h + 1) * 64].T.astype(bf))
                        for h in range(2)])
        in_maps.append({"xT": xT, "wqT": wqT, "wkT": wkT, "wvT": wvT, "woT": woT})
    return in_maps


def kernel(x, Wq, Wk, Wv, Wo, bo):
    global _NC_CACHE
    bo = np.asarray(bo, np.float32)
    in_maps = build_in_maps(x, Wq, Wk, Wv, Wo)

    if _NC_CACHE is None:
        _NC_CACHE = build_bass()
    res = run_bass_kernel_spmd(_NC_CACHE, in_maps, list(range(8)))
    partials = [np.asarray(res.results[c]["out"], np.float32) for c in range(8)]

    out = np.empty((B, N, D), np.float32)
    for b in range(B):
        out[b] = partials[4 * b] + partials[4 * b + 1] + partials[4 * b + 2] + partials[4 * b + 3] + bo
    return out


if __name__ == "__main__":
    nc = build_bass()
    print("built ok")



# revision 33
# speedup vs baseline: 2.2111x; 2.2111x over previous
"""Self-attention (8 heads, d=64, B=2, N=4096, D=512) on 8 TRN2 NeuronCores.

Sharding: batch*heads across cores — core c handles batch b=c//4, heads
(2*(c%4), 2*(c%4)+1). Projection weights are sliced per-core on the host;
x is pre-transposed on the host so the device needs no transposes at all.

Device dataflow (per core, "scoresT" formulation, v2 carries ones columns
so the softmax denominator falls out of the AV matmul):
  qT2/kT2 [hd=128, n]  = W.T-chunks @ xT-chunks            (PE)
  v2      [n, 65*2]    natural, ones at cols 64/129        (PE)
  per q-chunk qq (512 wide), per kc in 32, per head h:
    scT psum[128k, 512q] = kh.T @ qh                       (PE)
    attnT = exp(scT*SCALE) -> bf16 SBUF
      kc in EXP_ACT :  ACT table exp                       (ACT)
      kc in EXP_DVE :  Schraudolph int16 bit-hack          (DVE)
      kc in EXP_POOL:  Schraudolph int16 bit-hack          (Pool/gpsimd)
    av[65,512] += v2'[kc].T @ attnT                        (PE, accumulate)
  row 64 of av = softmax denominator; normalize via
    recip (DVE) -> PE K=1 broadcast matmul -> psum*psum mul (DVE)
  outT[h] slices feed the output projection (deferred by 2 kc into the
  next qq so PE never stalls on the normalize chain).
Host: out[b] = sum of its 4 cores' partials + bo.

The exp split keeps the Activation engine off the critical path (PE-paced);
Schraudolph exp = one tensor_scalar (x*A+B -> truncating int16 convert,
bitcast bf16), magic constant calibrated for truncation (max rel ~4%),
applied to 8/32 key-chunks -> end-to-end rel err ~1e-2 (budget 2e-2).
"""
import numpy as np
import ml_dtypes
from contextlib import ExitStack

import concourse.bass as bass
from concourse import bacc
import concourse.mybir as mybir
import concourse.tile as tile
from concourse.bass_utils import run_bass_kernel_spmd

B, N, D = 2, 4096, 512
HEADS, DH = 8, 64
SCALE = DH ** -0.5

F32 = mybir.dt.float32
F32R = mybir.dt.float32r
BF16 = mybir.dt.bfloat16
I16 = mybir.dt.int16

QQ_W = 512           # q-chunk width in the attention loop
N_QQ = N // QQ_W     # 8
N_KC = N // 128      # 32 key chunks
DCH = D // 128       # 4 contraction chunks for projections

# Schraudolph exp (truncating fp32->int16 convert, bitcast bf16)
LOG2E = 1.4426950408889634
SCH_A = 128.0 * LOG2E          # exponent-bit slope for bf16
SCH_B = 127.0 * 128.0 - 7.0    # magic constant calibrated for truncation

# kc's whose exp runs on DVE (Schraudolph). Every 3rd kc, so the ACT engine
# never runs 3 exps back-to-back — 3-bursts drift past the 2-deep score-psum
# ring recycle window and stall PE ~120ns per kc.
EXP_DVE = {2, 5, 8, 11, 14, 17, 20, 23, 26, 29}
EXP_POOL = set()                           # (gpsimd exp latency stalls the
                                           # sc psum ring; keep Pool out)


def build_bass():
    nc = bacc.Bacc(None, target_bir_lowering=False)

    xT = nc.dram_tensor("xT", [D, N], BF16, kind="ExternalInput")
    wqT = nc.dram_tensor("wqT", [D, 128], BF16, kind="ExternalInput")
    wkT = nc.dram_tensor("wkT", [D, 128], BF16, kind="ExternalInput")
    wvT = nc.dram_tensor("wvT", [D, 128], BF16, kind="ExternalInput")
    woT = nc.dram_tensor("woT", [2, 64, D], BF16, kind="ExternalInput")
    out = nc.dram_tensor("out", [N, D], F32, kind="ExternalOutput")
    recip_dram = nc.dram_tensor("recip_scratch", [N_QQ, 2, QQ_W], F32)

    with tile.TileContext(nc) as tc, ExitStack() as ctx:
        const = ctx.enter_context(tc.tile_pool(name="const", bufs=1))

        # ---- load inputs (small weights first; xT in fine chunks so the
        # projection matmuls start ~2us in and never starve) ----
        xT_sb = const.tile([128, DCH, N], BF16)            # xT[(c p), n] -> [p, c, n]
        xT_ap = xT.rearrange("(c p) n -> p c n", p=128)
        wq_sb = const.tile([128, DCH, 128], BF16)
        nc.sync.dma_start(out=wq_sb, in_=wqT.rearrange("(c p) m -> p c m", p=128))
        for c in range(DCH):
            nc.sync.dma_start(out=xT_sb[:, c, bass.ts(0, N // 8)],
                              in_=xT_ap[:, c, bass.ts(0, N // 8)])
        wk_sb = const.tile([128, DCH, 128], BF16)
        nc.sync.dma_start(out=wk_sb, in_=wkT.rearrange("(c p) m -> p c m", p=128))
        wv_sb = const.tile([128, DCH, 128], BF16)
        nc.sync.dma_start(out=wv_sb, in_=wvT.rearrange("(c p) m -> p c m", p=128))
        wo_sb = const.tile([64, 2, D], BF16)
        nc.sync.dma_start(out=wo_sb, in_=woT.rearrange("h d n -> d h n"))
        for i in range(1, 8):
            nc.sync.dma_start(out=xT_sb[:, :, bass.ts(i, N // 8)],
                              in_=xT_ap[:, :, bass.ts(i, N // 8)])

        qT2 = const.tile([128, N], BF16)                   # [2-head d, n]
        kT2 = const.tile([128, N], BF16)
        v2 = const.tile([128, N_KC, 130], BF16)            # [k-part, kc, (v_h0|1|v_h1|1)]
        outT = const.tile([64, 2, N], BF16)                # normalized per-head av

        # ones columns for the softmax-denominator trick (the v copies never
        # touch columns 64/129, so these can run before the projections)
        nc.vector.memset(v2[:, :, 64], 1.0)
        nc.vector.memset(v2[:, :, 129], 1.0)

        # ---- q/k projections, interleaved per n-tile so each 512-wide xT
        # chunk is consumed as it lands (psum->sbuf copies on ACT, idle here).
        # The v projection is deferred into qq0's attention loop: scores only
        # need qT2/kT2, and v2[kc] isn't consumed until the AV matmul. ----
        with tc.tile_pool(name="proj_psum", bufs=2, space="PSUM") as proj_psum:
            for nt in range(N // 512):
                pq = proj_psum.tile([128, 512], F32, tag="pj")
                for c in range(DCH):
                    nc.tensor.matmul(pq, wq_sb[:, c, :], xT_sb[:, c, bass.ts(nt, 512)],
                                     start=(c == 0), stop=(c == DCH - 1))
                nc.scalar.copy(qT2[:, bass.ts(nt, 512)], pq)
                pk = proj_psum.tile([128, 512], F32, tag="pj")
                for c in range(DCH):
                    nc.tensor.matmul(pk, wk_sb[:, c, :], xT_sb[:, c, bass.ts(nt, 512)],
                                     start=(c == 0), stop=(c == DCH - 1))
                nc.scalar.copy(kT2[:, bass.ts(nt, 512)], pk)

        # ---- attention ----
        with (
            tc.tile_pool(name="sc_psum", bufs=2, space="PSUM") as sc_psum,
            tc.tile_pool(name="av_psum", bufs=2, space="PSUM") as av_psum,
            tc.tile_pool(name="aux_psum", bufs=2, space="PSUM") as aux_psum,
            tc.tile_pool(name="attn_sb", bufs=8) as attn_sb,
            tc.tile_pool(name="norm_sb", bufs=2) as norm_sb,
            tc.tile_pool(name="ob_sb", bufs=2) as ob_sb,
        ):
            pending_norm = [None]        # (qq, avs) whose normalize is deferred
            pending_oproj = [None]

            def emit_vproj(nt):
                # v natural: out[n-tile, hd] = xT-chunk.T @ wv-chunk, through
                # the shared aux psum ring (viewed as 4 x [128,128] outputs)
                pv = aux_psum.tile([128, D], F32, tag="aux", name=f"pv_{nt}")
                for i in range(4):
                    kc = 4 * nt + i
                    for c in range(DCH):
                        nc.tensor.matmul(pv[:, bass.ts(i, 128)],
                                         xT_sb[:, c, bass.ts(kc, 128)],
                                         wv_sb[:, c, :],
                                         start=(c == 0), stop=(c == DCH - 1))
                # interleave the two heads' halves into v2 via strided APs
                for half, (off, dst0, dst1) in enumerate(((0, 0, 64), (64, 65, 129))):
                    src = pv[:, off:off + 64]
                    src3 = bass.AP(tensor=src.tensor, offset=src.offset,
                                   ap=[src.ap[0], [128, 4], [1, 64]])
                    nc.vector.tensor_copy(v2[:, 4 * nt:4 * nt + 4, dst0:dst1], src3)

            def emit_norm_recip(qq, avs):
                # 1/av[64] -> DRAM -> partition-broadcast back into SBUF. Only
                # the DMAs are in flight here; the muls wait until the
                # broadcast has landed so DVE never blocks head-of-line.
                bcs = []
                for h in range(2):
                    av = avs[h]
                    rc = norm_sb.tile([128, QQ_W], F32, tag="rc", name=f"rc_{qq}_{h}")
                    nc.vector.reciprocal(rc[64:65, :], av[64:65, :])
                    nc.sync.dma_start(out=recip_dram[qq:qq + 1, h, :],
                                      in_=rc[64:65, :])
                    bc = norm_sb.tile([64, QQ_W], F32, tag="bc", name=f"bc_{qq}_{h}")
                    src = recip_dram[qq, h, :]
                    bcast = bass.AP(tensor=src.tensor, offset=src.offset,
                                    ap=[[0, 64]] + src.ap)
                    nc.sync.dma_start(out=bc, in_=bcast)
                    bcs.append(bc)
                return bcs

            def emit_norm_mul(qq, avs, bcs):
                for h in range(2):
                    nc.vector.tensor_mul(outT[:, h, bass.ts(qq, QQ_W)],
                                         avs[h][0:64, :], bcs[h])

            def emit_oproj(qq):
                for j in range(QQ_W // 128):
                    nt = qq * (QQ_W // 128) + j
                    po = aux_psum.tile([128, D], F32, tag="aux", name=f"po_{nt}")
                    nc.tensor.matmul(po, outT[:, 0, bass.ts(nt, 128)], wo_sb[:, 0, :],
                                     start=True, stop=False)
                    nc.tensor.matmul(po, outT[:, 1, bass.ts(nt, 128)], wo_sb[:, 1, :],
                                     start=False, stop=True)
                    ob = ob_sb.tile([128, D], F32, tag="ob", name=f"ob_{nt}")
                    # copy on ACT: DVE copies here would delay its Schraudolph
                    # exps, which stalls the sc-ring recycle on PE
                    nc.scalar.copy(ob, po)
                    nc.sync.dma_start(out=out[bass.ts(nt, 128), :], in_=ob)

            for qq in range(N_QQ):
                avs = [av_psum.tile([65, QQ_W], F32, tag="av", name=f"av_{qq}_{h}")
                       for h in range(2)]
                pending_av = []          # (kc, at2) not yet fed to the AV matmul
                for kc in range(N_KC):
                    # scores for kc: both heads into one 2-bank psum tile so a
                    # single wide exp instruction covers them (halves the
                    # fixed access-latency overhead per element)
                    sc2 = sc_psum.tile([128, 2, QQ_W], F32, tag="sc",
                                       name=f"sc_{qq}_{kc}")
                    for h in range(2):
                        nc.tensor.matmul(
                            sc2[:, h, :],
                            kT2[h * 64:(h + 1) * 64, bass.ts(kc, 128)],
                            qT2[h * 64:(h + 1) * 64, bass.ts(qq, QQ_W)],
                            start=True, stop=True)
                    at2 = attn_sb.tile([128, 2, QQ_W], BF16, tag="at",
                                       name=f"at_{qq}_{kc}")
                    if kc in EXP_DVE or kc in EXP_POOL:
                        eng = nc.vector if kc in EXP_DVE else nc.gpsimd
                        eng.tensor_scalar(at2.bitcast(I16), sc2,
                                          float(SCALE * SCH_A), float(SCH_B),
                                          mybir.AluOpType.mult,
                                          mybir.AluOpType.add)
                    else:
                        nc.scalar.activation(at2, sc2,
                                             mybir.ActivationFunctionType.Exp,
                                             scale=float(SCALE))
                    # AV lags scores by 3 kc so exp latency never stalls PE
                    pending_av.append((kc, at2))
                    if len(pending_av) > 3:
                        pkc, pats = pending_av.pop(0)
                        for h in range(2):
                            nc.tensor.matmul(
                                avs[h], v2[:, pkc, h * 65:(h + 1) * 65], pats[:, h, :],
                                start=(pkc == 0), stop=False)
                    # deferred v projection rides inside qq0's loop
                    if qq == 0 and kc < N // 512:
                        emit_vproj(kc)
                    # previous qq's normalize, then its output projection, are
                    # deferred here so PE never waits on the DVE chain
                    if kc == 2 and pending_norm[0] is not None:
                        pq_, pavs_ = pending_norm[0]
                        pending_norm[0] = (pq_, pavs_, emit_norm_recip(pq_, pavs_))
                    if kc == 8 and pending_norm[0] is not None:
                        emit_norm_mul(*pending_norm[0])
                        pending_norm[0] = None
                    if kc == 11 and pending_oproj[0] is not None:
                        emit_oproj(pending_oproj[0])
                        pending_oproj[0] = None
                for pkc, pats in pending_av:
                    for h in range(2):
                        nc.tensor.matmul(avs[h], v2[:, pkc, h * 65:(h + 1) * 65],
                                         pats[:, h, :],
                                         start=(pkc == 0), stop=(pkc == N_KC - 1))
                pending_norm[0] = (qq, avs)
                pending_oproj[0] = qq
            qq_, avs_ = pending_norm[0]
            emit_norm_mul(qq_, avs_, emit_norm_recip(qq_, avs_))
            emit_oproj(pending_oproj[0])

    nc.compile()
    return nc


_NC_CACHE = None


def build_in_maps(x, Wq, Wk, Wv, Wo):
    bf = ml_dtypes.bfloat16
    x = np.asarray(x, np.float32)
    Wq, Wk, Wv, Wo = (np.asarray(a, np.float32) for a in (Wq, Wk, Wv, Wo))
    in_maps = []
    for c in range(8):
        b = c // 4
        h0 = 2 * (c % 4)
        xT = np.ascontiguousarray(x[b].T.astype(bf))
        wqT = np.ascontiguousarray(Wq[h0 * 64:(h0 + 2) * 64].T.astype(bf))
        wkT = np.ascontiguousarray(Wk[h0 * 64:(h0 + 2) * 64].T.astype(bf))
        wvT = np.ascontiguousarray(Wv[h0 * 64:(h0 + 2) * 64].T.astype(bf))
        woT = np.stack([np.ascontiguousarray(Wo[:, (h0 + h) * 64:(h0 + h + 1) * 64].T.astype(bf))
                        for h in range(2)])
        in_maps.append({"xT": xT, "wqT": wqT, "wkT": wkT, "wvT": wvT, "woT": woT})
    return in_maps


def kernel(x, Wq, Wk, Wv, Wo, bo):
    global _NC_CACHE
    bo = np.asarray(bo, np.float32)
    in_maps = build_in_maps(x, Wq, Wk, Wv, Wo)

    if _NC_CACHE is None:
        _NC_CACHE = build_bass()
    res = run_bass_kernel_spmd(_NC_CACHE, in_maps, list(range(8)))
    partials = [np.asarray(res.results[c]["out"], np.float32) for c in range(8)]

    out = np.empty((B, N, D), np.float32)
    for b in range(B):
        out[b] = partials[4 * b] + partials[4 * b + 1] + partials[4 * b + 2] + partials[4 * b + 3] + bo
    return out


if __name__ == "__main__":
    nc = build_bass()
    print("built ok")


# revision 39
# speedup vs baseline: 2.3992x; 1.0851x over previous
"""Self-attention (8 heads, d=64, B=2, N=4096, D=512) on 8 TRN2 NeuronCores.

Sharding: batch*heads across cores — core c handles batch b=c//4, heads
(2*(c%4), 2*(c%4)+1). Projection weights are sliced per-core on the host;
x is pre-transposed on the host so the device needs no transposes at all.

Device dataflow (per core, "scoresT" formulation, v2 carries ones columns
so the softmax denominator falls out of the AV matmul):
  qT2/kT2 [hd=128, n]  = W.T-chunks @ xT-chunks            (PE)
  v2      [n, 65*2]    natural, ones at cols 64/129        (PE)
  per q-chunk qq (512 wide), per kc in 32, per head h:
    scT psum[128k, 512q] = kh.T @ qh                       (PE)
    attnT = exp(scT*SCALE) -> bf16 SBUF
      kc in EXP_ACT :  ACT table exp                       (ACT)
      kc in EXP_DVE :  Schraudolph int16 bit-hack          (DVE)
      kc in EXP_POOL:  Schraudolph int16 bit-hack          (Pool/gpsimd)
    av[65,512] += v2'[kc].T @ attnT                        (PE, accumulate)
  row 64 of av = softmax denominator; normalize via
    recip (DVE) -> PE K=1 broadcast matmul -> psum*psum mul (DVE)
  outT[h] slices feed the output projection (deferred by 2 kc into the
  next qq so PE never stalls on the normalize chain).
Host: out[b] = sum of its 4 cores' partials + bo.

The exp split keeps the Activation engine off the critical path (PE-paced);
Schraudolph exp = one tensor_scalar (x*A+B -> truncating int16 convert,
bitcast bf16), magic constant calibrated for truncation (max rel ~4%),
applied to 8/32 key-chunks -> end-to-end rel err ~1e-2 (budget 2e-2).
"""
import numpy as np
import ml_dtypes
from contextlib import ExitStack

import concourse.bass as bass
from concourse import bacc
import concourse.mybir as mybir
import concourse.tile as tile
from concourse.bass_utils import run_bass_kernel_spmd

B, N, D = 2, 4096, 512
HEADS, DH = 8, 64
SCALE = DH ** -0.5

F32 = mybir.dt.float32
F32R = mybir.dt.float32r
BF16 = mybir.dt.bfloat16
I16 = mybir.dt.int16

QQ_W = 512           # q-chunk width in the attention loop
N_QQ = N // QQ_W     # 8
N_KC = N // 128      # 32 key chunks
DCH = D // 128       # 4 contraction chunks for projections

# Schraudolph exp (truncating fp32->int16 convert, bitcast bf16)
LOG2E = 1.4426950408889634
SCH_A = 128.0 * LOG2E          # exponent-bit slope for bf16
SCH_B = 127.0 * 128.0 - 7.0    # magic constant calibrated for truncation

# kc's whose exp runs on DVE (Schraudolph). Every 3rd kc INCLUDING kc 31, so
# ACT never runs 3+ exps back-to-back even across the qq wraparound — exp
# bursts drift past the 2-deep score-psum ring recycle window and stall PE.
EXP_DVE = set(range(1, N_KC, 3))
EXP_POOL = set()                           # (gpsimd exp latency stalls the
                                           # sc psum ring; keep Pool out)


def build_bass():
    nc = bacc.Bacc(None, target_bir_lowering=False)

    xT = nc.dram_tensor("xT", [D, N], BF16, kind="ExternalInput")
    wqT = nc.dram_tensor("wqT", [D, 128], BF16, kind="ExternalInput")
    wkT = nc.dram_tensor("wkT", [D, 128], BF16, kind="ExternalInput")
    wvT = nc.dram_tensor("wvT", [D, 128], BF16, kind="ExternalInput")
    woT = nc.dram_tensor("woT", [2, 64, D], BF16, kind="ExternalInput")
    out = nc.dram_tensor("out", [N, D], F32, kind="ExternalOutput")
    recip_dram = nc.dram_tensor("recip_scratch", [N_QQ, 2, QQ_W], F32)

    with tile.TileContext(nc) as tc, ExitStack() as ctx:
        const = ctx.enter_context(tc.tile_pool(name="const", bufs=1))

        # ---- load inputs (small weights first; xT in fine chunks so the
        # projection matmuls start ~2us in and never starve) ----
        xT_sb = const.tile([128, DCH, N], BF16)            # xT[(c p), n] -> [p, c, n]
        xT_ap = xT.rearrange("(c p) n -> p c n", p=128)
        wq_sb = const.tile([128, DCH, 128], BF16)
        nc.sync.dma_start(out=wq_sb, in_=wqT.rearrange("(c p) m -> p c m", p=128))
        for c in range(DCH):
            nc.sync.dma_start(out=xT_sb[:, c, bass.ts(0, N // 8)],
                              in_=xT_ap[:, c, bass.ts(0, N // 8)])
        wk_sb = const.tile([128, DCH, 128], BF16)
        nc.sync.dma_start(out=wk_sb, in_=wkT.rearrange("(c p) m -> p c m", p=128))
        wv_sb = const.tile([128, DCH, 128], BF16)
        nc.sync.dma_start(out=wv_sb, in_=wvT.rearrange("(c p) m -> p c m", p=128))
        wo_sb = const.tile([64, 2, D], BF16)
        nc.sync.dma_start(out=wo_sb, in_=woT.rearrange("h d n -> d h n"))
        for i in range(1, 8):
            nc.sync.dma_start(out=xT_sb[:, :, bass.ts(i, N // 8)],
                              in_=xT_ap[:, :, bass.ts(i, N // 8)])

        qT2 = const.tile([128, N], BF16)                   # [2-head d, n]
        kT2 = const.tile([128, N], BF16)
        v2 = const.tile([128, N_KC, 130], BF16)            # [k-part, kc, (v_h0|1|v_h1|1)]
        outT = const.tile([64, 2, N], BF16)                # normalized per-head av
        ones_bc = const.tile([128, 64], BF16)              # lane 64 used as bcast lhsT
        nc.vector.memset(ones_bc, 1.0)

        # ones columns for the softmax-denominator trick (the v copies never
        # touch columns 64/129, so these can run before the projections)
        nc.vector.memset(v2[:, :, 64], 1.0)
        nc.vector.memset(v2[:, :, 129], 1.0)

        # ---- q/k projections, interleaved per n-tile so each 512-wide xT
        # chunk is consumed as it lands (psum->sbuf copies on ACT, idle here).
        # The v projection is deferred into qq0's attention loop: scores only
        # need qT2/kT2, and v2[kc] isn't consumed until the AV matmul. ----
        with tc.tile_pool(name="proj_psum", bufs=2, space="PSUM") as proj_psum:
            for nt in range(N // 512):
                pq = proj_psum.tile([128, 512], F32, tag="pj")
                for c in range(DCH):
                    nc.tensor.matmul(pq, wq_sb[:, c, :], xT_sb[:, c, bass.ts(nt, 512)],
                                     start=(c == 0), stop=(c == DCH - 1))
                nc.scalar.copy(qT2[:, bass.ts(nt, 512)], pq)
                pk = proj_psum.tile([128, 512], F32, tag="pj")
                for c in range(DCH):
                    nc.tensor.matmul(pk, wk_sb[:, c, :], xT_sb[:, c, bass.ts(nt, 512)],
                                     start=(c == 0), stop=(c == DCH - 1))
                nc.scalar.copy(kT2[:, bass.ts(nt, 512)], pk)

        # ---- attention ----
        with (
            tc.tile_pool(name="sc_psum", bufs=2, space="PSUM") as sc_psum,
            tc.tile_pool(name="av_psum", bufs=2, space="PSUM") as av_psum,
            tc.tile_pool(name="aux_psum", bufs=2, space="PSUM") as aux_psum,
            tc.tile_pool(name="attn_sb", bufs=8) as attn_sb,
            tc.tile_pool(name="norm_sb", bufs=2) as norm_sb,
            tc.tile_pool(name="ob_sb", bufs=2) as ob_sb,
        ):
            pending_norm = [None]        # (qq, avs) whose normalize is deferred
            pending_oproj = [None]

            def emit_vproj(nt):
                # v natural: out[n-tile, hd] = xT-chunk.T @ wv-chunk, through
                # the shared aux psum ring (viewed as 4 x [128,128] outputs)
                pv = aux_psum.tile([128, D], F32, tag="aux", name=f"pv_{nt}")
                for i in range(4):
                    kc = 4 * nt + i
                    for c in range(DCH):
                        nc.tensor.matmul(pv[:, bass.ts(i, 128)],
                                         xT_sb[:, c, bass.ts(kc, 128)],
                                         wv_sb[:, c, :],
                                         start=(c == 0), stop=(c == DCH - 1))
                # interleave the two heads' halves into v2 via strided APs
                for half, (off, dst0, dst1) in enumerate(((0, 0, 64), (64, 65, 129))):
                    src = pv[:, off:off + 64]
                    src3 = bass.AP(tensor=src.tensor, offset=src.offset,
                                   ap=[src.ap[0], [128, 4], [1, 64]])
                    nc.vector.tensor_copy(v2[:, 4 * nt:4 * nt + 4, dst0:dst1], src3)

            def emit_norm_head(qq, avs, h):
                # outT[h][:, qq] = av[:64] * (1/av[64]): reciprocal row (fp32),
                # bf16 convert, K=1 PE broadcast matmul from lane 64, bounce the
                # psum broadcast to SBUF, then a one-psum-operand multiply.
                av = avs[h]
                rc = norm_sb.tile([128, QQ_W], F32, tag="rc", name=f"rc_{qq}_{h}")
                nc.vector.reciprocal(rc[64:65, :], av[64:65, :])
                rcb = norm_sb.tile([128, QQ_W], BF16, tag="rcb", name=f"rcb_{qq}_{h}")
                nc.vector.tensor_copy(rcb[64:65, :], rc[64:65, :])
                bc = aux_psum.tile([128, D], F32, tag="aux", name=f"bc_{qq}_{h}")
                nc.tensor.matmul(bc[0:64, :], ones_bc[64:65, :], rcb[64:65, :],
                                 start=True, stop=True)
                bcs = norm_sb.tile([64, QQ_W], F32, tag="bcs", name=f"bcs_{qq}_{h}")
                nc.vector.tensor_copy(bcs, bc[0:64, :])
                nc.vector.tensor_mul(outT[:, h, bass.ts(qq, QQ_W)],
                                     av[0:64, :], bcs)

            def emit_oproj_one(qq, j):
                nt = qq * (QQ_W // 128) + j
                po = aux_psum.tile([128, D], F32, tag="aux", name=f"po_{nt}")
                nc.tensor.matmul(po, outT[:, 0, bass.ts(nt, 128)], wo_sb[:, 0, :],
                                 start=True, stop=False)
                nc.tensor.matmul(po, outT[:, 1, bass.ts(nt, 128)], wo_sb[:, 1, :],
                                 start=False, stop=True)
                ob = ob_sb.tile([128, D], F32, tag="ob", name=f"ob_{nt}")
                # copy on ACT (spread one per 3 kc so its exp stream absorbs
                # them); DVE copies here would delay the Schraudolph exps
                nc.scalar.copy(ob, po)
                nc.sync.dma_start(out=out[bass.ts(nt, 128), :], in_=ob)

            for qq in range(N_QQ):
                avs = [av_psum.tile([65, QQ_W], F32, tag="av", name=f"av_{qq}_{h}")
                       for h in range(2)]
                pending_av = []          # (kc, at2) not yet fed to the AV matmul
                for kc in range(N_KC):
                    # scores for kc: both heads into one 2-bank psum tile so a
                    # single wide exp instruction covers them (halves the
                    # fixed access-latency overhead per element)
                    sc2 = sc_psum.tile([128, 2, QQ_W], F32, tag="sc",
                                       name=f"sc_{qq}_{kc}")
                    for h in range(2):
                        nc.tensor.matmul(
                            sc2[:, h, :],
                            kT2[h * 64:(h + 1) * 64, bass.ts(kc, 128)],
                            qT2[h * 64:(h + 1) * 64, bass.ts(qq, QQ_W)],
                            start=True, stop=True)
                    at2 = attn_sb.tile([128, 2, QQ_W], BF16, tag="at",
                                       name=f"at_{qq}_{kc}")
                    if kc in EXP_DVE or kc in EXP_POOL:
                        eng = nc.vector if kc in EXP_DVE else nc.gpsimd
                        eng.tensor_scalar(at2.bitcast(I16), sc2,
                                          float(SCALE * SCH_A), float(SCH_B),
                                          mybir.AluOpType.mult,
                                          mybir.AluOpType.add)
                    else:
                        nc.scalar.activation(at2, sc2,
                                             mybir.ActivationFunctionType.Exp,
                                             scale=float(SCALE))
                    # AV lags scores by 3 kc so exp latency never stalls PE
                    pending_av.append((kc, at2))
                    if len(pending_av) > 3:
                        pkc, pats = pending_av.pop(0)
                        for h in range(2):
                            nc.tensor.matmul(
                                avs[h], v2[:, pkc, h * 65:(h + 1) * 65], pats[:, h, :],
                                start=(pkc == 0), stop=False)
                    # deferred v projection rides inside qq0's loop
                    if qq == 0 and kc < N // 512:
                        emit_vproj(kc)
                    # previous qq's normalize, then its output projection, are
                    # deferred here so PE never waits on the DVE chain
                    if pending_norm[0] is not None:
                        # kc 2 and 5 sit between DVE exp kcs (1,4,7), so the
                        # normalize's DVE chain never delays a Schraudolph exp
                        if kc == 2:
                            emit_norm_head(*pending_norm[0], 0)
                        elif kc == 5:
                            emit_norm_head(*pending_norm[0], 1)
                            pending_norm[0] = None
                    if pending_oproj[0] is not None and kc in (9, 12, 15, 18):
                        emit_oproj_one(pending_oproj[0], (kc - 9) // 3)
                        if kc == 18:
                            pending_oproj[0] = None
                for pkc, pats in pending_av:
                    for h in range(2):
                        nc.tensor.matmul(avs[h], v2[:, pkc, h * 65:(h + 1) * 65],
                                         pats[:, h, :],
                                         start=(pkc == 0), stop=(pkc == N_KC - 1))
                pending_norm[0] = (qq, avs)
                pending_oproj[0] = qq
            qq_, avs_ = pending_norm[0]
            emit_norm_head(qq_, avs_, 0)
            emit_norm_head(qq_, avs_, 1)
            for j in range(QQ_W // 128):
                emit_oproj_one(pending_oproj[0], j)

    nc.compile()
    return nc


_NC_CACHE = None


def build_in_maps(x, Wq, Wk, Wv, Wo):
    bf = ml_dtypes.bfloat16
    x = np.asarray(x, np.float32)
    Wq, Wk, Wv, Wo = (np.asarray(a, np.float32) for a in (Wq, Wk, Wv, Wo))
    in_maps = []
    for c in range(8):
        b = c // 4
        h0 = 2 * (c % 4)
        xT = np.ascontiguousarray(x[b].T.astype(bf))
        wqT = np.ascontiguousarray(Wq[h0 * 64:(h0 + 2) * 64].T.astype(bf))
        wkT = np.ascontiguousarray(Wk[h0 * 64:(h0 + 2) * 64].T.astype(bf))
        wvT = np.ascontiguousarray(Wv[h0 * 64:(h0 + 2) * 64].T.astype(bf))
        woT = np.stack([np.ascontiguousarray(Wo[:, (h0 + h) * 64:(h0 + h + 1) * 64].T.astype(bf))
                        for h in range(2)])
        in_maps.append({"xT": xT, "wqT": wqT, "wkT": wkT, "wvT": wvT, "woT": woT})
    return in_maps


def kernel(x, Wq, Wk, Wv, Wo, bo):
    global _NC_CACHE
    bo = np.asarray(bo, np.float32)
    in_maps = build_in_maps(x, Wq, Wk, Wv, Wo)

    if _NC_CACHE is None:
        _NC_CACHE = build_bass()
    res = run_bass_kernel_spmd(_NC_CACHE, in_maps, list(range(8)))
    partials = [np.asarray(res.results[c]["out"], np.float32) for c in range(8)]

    out = np.empty((B, N, D), np.float32)
    for b in range(B):
        out[b] = partials[4 * b] + partials[4 * b + 1] + partials[4 * b + 2] + partials[4 * b + 3] + bo
    return out


if __name__ == "__main__":
    nc = build_bass()
    print("built ok")
